# revision 26
# baseline (speedup 1.0000x reference)
"""Trainium2 Bass kernel for nn_Encoder (embedding lookup + masked LSTM scan).

Reference semantics (Keras-style masked LSTM, mask_zero=True):
    x      = emb[tokens]                      # [B,T,E]
    z_t    = x_t @ W + b + h_{t-1} @ U        # [B,4H], gates i,f,g,o
    c_t    = sigmoid(f)*c_{t-1} + sigmoid(i)*tanh(g)
    h_t    = sigmoid(o)*tanh(c_t)
    masked steps (token==0) carry h,c; output copies previous output.

Device strategy (data-parallel over batch, 8 cores x 64 rows):
  * Layout "B": state and gates live transposed [H, b] so the recurrence
    matmul z^T = lhsT^T @ [h; x; 1; (1-m)] needs no per-step transpose.
  * The input projection W, bias b and the mask logic are FUSED into the
    recurrence matmul: lhsT rows 0:64 = U, 64:114 = W, 114 = b,
    115 = +-BIG mask logit offsets. rhs rows 64:128 are gathered columns
    of the (host-augmented) embedding table: a constant-1 column (bias)
    and a (v==0) column (mask). At masked steps the offsets force
    i=0, f=1, o=0 so c carries and h2 = 0 + (1-m)*h carries.
  * A 5th matmul column block replicates the (1-m) rhs row across the 64
    output partitions, giving the h-carry multiplier as a full tile.
  * Embedding gather: SWDGE dma_gather (1024 rows/call - firmware limit;
    2048 crashes the exec unit), PE-transposed in [128,64] chunks, then
    crossbar-DMA'd into the x half of the rhs staging. All prefetched
    ahead of the scan, off the critical path.
  * The rhs staging is append-only (block t = [hT_t; x_t], each written
    exactly once) -> no WAR hazards anywhere; h history rows double as
    the output, bulk-DMA'd at the end.
  * Matmul emission order g,i,f,o,m lets tanh(g) run on ACT before the
    wide sigmoid, shortening the serial chain.
"""

import sys

import numpy as np

sys.path.insert(0, "/opt/trn_rl_repo")

B, T, V, E, H = 512, 512, 17067, 50, 64
NCORES = 8
BL = B // NCORES          # 64 batch rows per core
EP = 64                   # padded embedding row (256B)
G4 = 4 * H                # 256 gate columns
BIG = 100.0               # mask logit offset
P = 128
CH = 1024                 # tokens per dma_gather call (16 steps)

_CACHE = {}


def build_nc(t_steps=T, bl=BL, skip=()):
    """Build + compile the per-core Bass program (SPMD, identical on all cores)."""
    import concourse.bacc as bacc
    import concourse.mybir as mybir
    import concourse.tile as tile
    from concourse.library_config import mlp
    from concourse.masks import make_identity
    from contextlib import ExitStack

    f32 = mybir.dt.float32
    i16 = mybir.dt.int16
    AF = mybir.ActivationFunctionType

    n_idx = t_steps * bl
    chunk = min(CH, n_idx)
    n_chunks = n_idx // chunk

    nc = bacc.Bacc("TRN2", target_bir_lowering=False, num_devices=NCORES)

    emb_pad = nc.dram_tensor("emb_pad", [V, EP], f32, kind="ExternalInput")
    idx_w = nc.dram_tensor("idx_w", [P, n_idx // 16], i16, kind="ExternalInput")
    lhs_all = nc.dram_tensor("lhs_all", [P, 5 * 64], f32, kind="ExternalInput")
    h0T = nc.dram_tensor("h0T", [H, bl], f32, kind="ExternalInput")
    c0T = nc.dram_tensor("c0T", [H, bl], f32, kind="ExternalInput")
    outT = nc.dram_tensor("outT", [H, t_steps * bl], f32, kind="ExternalOutput")
    hTf = nc.dram_tensor("hTf", [H, bl], f32, kind="ExternalOutput")
    cTf = nc.dram_tensor("cTf", [H, bl], f32, kind="ExternalOutput")

    with tile.TileContext(nc) as tc, ExitStack() as ctx:
        consts = ctx.enter_context(tc.tile_pool(name="consts", bufs=1))
        idx_sb = consts.tile([P, n_idx // 16], i16)
        lhs_sb = consts.tile([P, 5 * 64], f32)
        # Full rhs staging: block t = [hT_t; x_t-aug], written exactly once
        # (x by the gather pipeline, hT_{t+1} by the scan) -> no WAR hazards.
        ring = consts.tile([P, (t_steps + 1) * bl], f32)
        ident = consts.tile([P, P], f32)
        make_identity(nc, ident[:])

        nc.sync.dma_start(idx_sb[:], idx_w.ap())
        nc.sync.dma_start(lhs_sb[:], lhs_all.ap())
        nc.sync.dma_start(ring[0:H, 0:bl], h0T.ap())

        cpool = ctx.enter_context(tc.tile_pool(name="cstate", bufs=1))
        c_tiles = [
            cpool.tile([H, bl], f32, tag=f"c{i}", name=f"cstate{i}") for i in range(2)
        ]
        nc.sync.dma_start(c_tiles[0][:], c0T.ap())

        # ---- embedding gather pipeline (fills x half of the staging) ----
        nc.gpsimd.load_library(mlp)
        gat = ctx.enter_context(tc.tile_pool(name="gather", bufs=2))
        trp = ctx.enter_context(tc.tile_pool(name="trp", bufs=2, space="PSUM"))
        xsg = ctx.enter_context(tc.tile_pool(name="xstage", bufs=3))
        sub = chunk // 128  # PE transposes per gather chunk
        for k in range(n_chunks):
            xr = gat.tile([P, sub, EP], f32, tag="xraw")
            if "gather" in skip:
                nc.vector.memset(xr[:], 0.0)
            else:
                nc.gpsimd.dma_gather(
                    xr[:],
                    emb_pad.ap(),
                    idx_sb[:, (chunk // 16) * k : (chunk // 16) * (k + 1)],
                    chunk,
                    chunk,
                    EP,
                )
            for c in range(sub):
                tp = trp.tile([EP, P], f32, tag="tp")
                nc.tensor.transpose(tp[:], xr[:, c, :], ident[:])
                xs = xsg.tile([EP, P], f32, tag="xs")
                nc.scalar.activation(xs[:], tp[:], AF.Copy)
                j0 = k * chunk + c * 128  # flat column position (no wrap)
                # crossbar DMA: partitions 0:64 -> 64:128 of the rhs staging
                nc.sync.dma_start(ring[64:128, j0 : j0 + 128], xs[:])

        # ---- the sequential scan ----
        psum = ctx.enter_context(tc.tile_pool(name="psum", bufs=2, space="PSUM"))
        sb = ctx.enter_context(tc.tile_pool(name="sb", bufs=2))
        for t in range(0 if "scan" in skip else t_steps):
            rb = t * bl
            rhs_t = ring[:, rb : rb + bl]
            ps_sig = psum.tile([H, 3 * bl], f32, tag="sig")
            ps_g = psum.tile([H, bl], f32, tag="g")
            ps_m = psum.tile([H, bl], f32, tag="m")
            # g first so tanh(g) can run on ACT ahead of the wide sigmoid
            nc.tensor.matmul(ps_g[:], lhs_sb[:, 128:192], rhs_t, start=True, stop=True)
            nc.tensor.matmul(ps_sig[:, 0:bl], lhs_sb[:, 0:64], rhs_t, start=True, stop=True)
            nc.tensor.matmul(ps_sig[:, bl : 2 * bl], lhs_sb[:, 64:128], rhs_t, start=True, stop=True)
            nc.tensor.matmul(ps_sig[:, 2 * bl : 3 * bl], lhs_sb[:, 192:256], rhs_t, start=True, stop=True)
            nc.tensor.matmul(ps_m[:], lhs_sb[:, 256:320], rhs_t, start=True, stop=True)

            gg = sb.tile([H, bl], f32, tag="gg")
            nc.scalar.activation(gg[:], ps_g[:], AF.Tanh)
            gsig = sb.tile([H, 3 * bl], f32, tag="gsig")
            nc.scalar.activation(gsig[:], ps_sig[:], AF.Sigmoid)

            # (1-m)*h first: ready as soon as the mask matmul lands, fills
            # the DVE while ACT is still busy
            tmp4 = sb.tile([H, bl], f32, tag="tmp4")
            nc.vector.tensor_mul(tmp4[:], ps_m[:], ring[0:H, rb : rb + bl])

            tmp = sb.tile([H, bl], f32, tag="tmp")
            nc.vector.tensor_mul(tmp[:], gsig[:, 0:bl], gg[:])            # i * tanh(g)
            c_old = c_tiles[t % 2]
            c_new = c_tiles[(t + 1) % 2]
            tmp2 = sb.tile([H, bl], f32, tag="tmp2")
            nc.vector.tensor_mul(tmp2[:], gsig[:, bl : 2 * bl], c_old[:])  # f * c
            nc.vector.tensor_add(c_new[:], tmp[:], tmp2[:])

            th = sb.tile([H, bl], f32, tag="th")
            nc.scalar.activation(th[:], c_new[:], AF.Tanh)

            tmp3 = sb.tile([H, bl], f32, tag="tmp3")
            nc.vector.tensor_mul(tmp3[:], gsig[:, 2 * bl : 3 * bl], th[:])  # o * tanh(c)
            nb = (t + 1) * bl
            nc.vector.tensor_add(ring[0:H, nb : nb + bl], tmp3[:], tmp4[:])

        # ---- outputs: h2 history = staging rows 0:64, blocks 1..T ----
        n_out_dma = 8 if t_steps % 8 == 0 else 1
        span = t_steps * bl // n_out_dma
        for d in range(n_out_dma):
            nc.sync.dma_start(
                outT.ap()[:, d * span : (d + 1) * span],
                ring[0:H, bl + d * span : bl + (d + 1) * span],
            )
        nc.sync.dma_start(hTf.ap(), ring[0:H, t_steps * bl : (t_steps + 1) * bl])
        nc.sync.dma_start(cTf.ap(), c_tiles[t_steps % 2][:])

    nc.compile()
    return nc


def host_inputs(tokens, h0, c0, emb, W, U, b, t_steps=T, bl=BL, ncores=NCORES):
    """Build per-core input maps (pure layout prep: pad/concat/transpose/cast)."""
    f32 = np.float32
    emb_pad = np.zeros((V, EP), f32)
    emb_pad[:, :E] = np.asarray(emb, f32)
    emb_pad[:, E] = 1.0          # bias/ones column
    emb_pad[:, E + 1] = 0.0      # one-minus-mask column
    emb_pad[0, E + 1] = 1.0

    lhs = np.zeros((P, 5 * 64), f32)
    lhs[0:H, 0:G4] = np.asarray(U, f32)
    lhs[H : H + E, 0:G4] = np.asarray(W, f32)
    lhs[H + E, 0:G4] = np.asarray(b, f32)
    lhs[H + E + 1, 0:H] = -BIG            # i
    lhs[H + E + 1, H : 2 * H] = BIG       # f
    lhs[H + E + 1, 2 * H : 3 * H] = 0.0   # g
    lhs[H + E + 1, 3 * H : 4 * H] = -BIG  # o
    lhs[H + E + 1, 4 * H : 5 * H] = 1.0   # mask replicator column block

    in_maps = []
    for cid in range(ncores):
        tk = np.asarray(tokens[cid * bl : (cid + 1) * bl, :t_steps])
        flat = tk.T.reshape(-1)  # t-major: n = t*bl + b
        idxw = flat.reshape(-1, 16).T.astype(np.int16)   # [16, n/16], n = col*16+row
        idx_full = np.tile(idxw, (8, 1))                 # [128, n/16]
        in_maps.append(
            {
                "emb_pad": emb_pad,
                "idx_w": np.ascontiguousarray(idx_full),
                "lhs_all": lhs,
                "h0T": np.ascontiguousarray(np.asarray(h0, f32)[cid * bl : (cid + 1) * bl].T),
                "c0T": np.ascontiguousarray(np.asarray(c0, f32)[cid * bl : (cid + 1) * bl].T),
            }
        )
    return in_maps


def host_assemble(results, tokens, t_steps=T, bl=BL, ncores=NCORES):
    """Un-transpose shards, apply the masked-output carry fix, concat."""
    outs, hts, cts = [], [], []
    for cid in range(ncores):
        r = results[cid]
        hist = r["outT"].reshape(H, t_steps, bl).transpose(2, 1, 0)  # [b,t,h]
        tk = np.asarray(tokens[cid * bl : (cid + 1) * bl, :t_steps])
        m = tk != 0
        last = np.maximum.accumulate(
            np.where(m, np.arange(t_steps)[None, :], -1), axis=1
        )
        gathered = hist[np.arange(bl)[:, None], np.clip(last, 0, None)]
        out = np.where((last >= 0)[:, :, None], gathered, 0.0).astype(np.float32)
        outs.append(out)
        hts.append(r["hTf"].T.copy())
        cts.append(r["cTf"].T.copy())
    return (
        np.concatenate(outs, 0),
        np.concatenate(hts, 0),
        np.concatenate(cts, 0),
    )


def kernel(tokens, h0, c0, emb, W, U, b):
    from concourse.bass_utils import run_bass_kernel_spmd

    if "nc" not in _CACHE:
        _CACHE["nc"] = build_nc()
    nc = _CACHE["nc"]
    in_maps = host_inputs(tokens, h0, c0, emb, W, U, b)
    res = run_bass_kernel_spmd(nc, in_maps, list(range(NCORES)))
    return host_assemble(res.results, tokens)


# revision 27
# speedup vs baseline: 1.0443x; 1.0443x over previous
"""Trainium2 Bass kernel for nn_Encoder (embedding lookup + masked LSTM scan).

Reference semantics (Keras-style masked LSTM, mask_zero=True):
    x      = emb[tokens]                      # [B,T,E]
    z_t    = x_t @ W + b + h_{t-1} @ U        # [B,4H], gates i,f,g,o
    c_t    = sigmoid(f)*c_{t-1} + sigmoid(i)*tanh(g)
    h_t    = sigmoid(o)*tanh(c_t)
    masked steps (token==0) carry h,c; output copies previous output.

Device strategy (data-parallel over batch, 8 cores x 64 rows):
  * Layout "B": state and gates live transposed [H, b] so the recurrence
    matmul z^T = lhsT^T @ [h; x; 1; (1-m)] needs no per-step transpose.
  * The input projection W, bias b and the mask logic are FUSED into the
    recurrence matmul: lhsT rows 0:64 = U, 64:114 = W, 114 = b,
    115 = +-BIG mask logit offsets. rhs rows 64:128 are gathered columns
    of the (host-augmented) embedding table: a constant-1 column (bias)
    and a (v==0) column (mask). At masked steps the offsets force
    i=0, f=1, o=0 so c carries and h2 = 0 + (1-m)*h carries.
  * A 5th matmul column block replicates the (1-m) rhs row across the 64
    output partitions, giving the h-carry multiplier as a full tile.
  * Embedding gather: SWDGE dma_gather (1024 rows/call - firmware limit;
    2048 crashes the exec unit), PE-transposed in [128,64] chunks, then
    crossbar-DMA'd into the x half of the rhs staging. All prefetched
    ahead of the scan, off the critical path.
  * The rhs staging is append-only (block t = [hT_t; x_t], each written
    exactly once) -> no WAR hazards anywhere; h history rows double as
    the output, bulk-DMA'd at the end.
  * Matmul emission order g,i,f,o,m lets tanh(g) run on ACT before the
    wide sigmoid, shortening the serial chain.
"""

import sys

import numpy as np

sys.path.insert(0, "/opt/trn_rl_repo")

B, T, V, E, H = 512, 512, 17067, 50, 64
NCORES = 8
BL = B // NCORES          # 64 batch rows per core
EP = 64                   # padded embedding row (256B)
G4 = 4 * H                # 256 gate columns
BIG = 100.0               # mask logit offset
P = 128
CH = 1024                 # tokens per dma_gather call (16 steps)

_CACHE = {}


def build_nc(t_steps=T, bl=BL, skip=()):
    """Build + compile the per-core Bass program (SPMD, identical on all cores)."""
    import concourse.bacc as bacc
    import concourse.mybir as mybir
    import concourse.tile as tile
    from concourse.library_config import mlp
    from concourse.masks import make_identity
    from contextlib import ExitStack

    f32 = mybir.dt.float32
    i16 = mybir.dt.int16
    AF = mybir.ActivationFunctionType

    n_idx = t_steps * bl
    chunk = min(CH, n_idx)
    n_chunks = n_idx // chunk

    nc = bacc.Bacc("TRN2", target_bir_lowering=False, num_devices=NCORES)

    emb_pad = nc.dram_tensor("emb_pad", [V, EP], f32, kind="ExternalInput")
    idx_w = nc.dram_tensor("idx_w", [P, n_idx // 16], i16, kind="ExternalInput")
    lhs_all = nc.dram_tensor("lhs_all", [P, 5 * 64], f32, kind="ExternalInput")
    h0T = nc.dram_tensor("h0T", [H, bl], f32, kind="ExternalInput")
    c0T = nc.dram_tensor("c0T", [H, bl], f32, kind="ExternalInput")
    outT = nc.dram_tensor("outT", [H, t_steps * bl], f32, kind="ExternalOutput")
    hTf = nc.dram_tensor("hTf", [H, bl], f32, kind="ExternalOutput")
    cTf = nc.dram_tensor("cTf", [H, bl], f32, kind="ExternalOutput")

    with tile.TileContext(nc) as tc, ExitStack() as ctx:
        consts = ctx.enter_context(tc.tile_pool(name="consts", bufs=1))
        idx_sb = consts.tile([P, n_idx // 16], i16)
        lhs_sb = consts.tile([P, 5 * 64], f32)
        # Full rhs staging: block t = [hT_t; x_t-aug], written exactly once
        # (x by the gather pipeline, hT_{t+1} by the scan) -> no WAR hazards.
        ring = consts.tile([P, (t_steps + 1) * bl], f32)
        ident = consts.tile([P, P], f32)
        make_identity(nc, ident[:])

        nc.sync.dma_start(idx_sb[:], idx_w.ap())
        nc.sync.dma_start(lhs_sb[:], lhs_all.ap())
        nc.sync.dma_start(ring[0:H, 0:bl], h0T.ap())

        cpool = ctx.enter_context(tc.tile_pool(name="cstate", bufs=1))
        c_tiles = [
            cpool.tile([H, bl], f32, tag=f"c{i}", name=f"cstate{i}") for i in range(2)
        ]
        nc.sync.dma_start(c_tiles[0][:], c0T.ap())

        # ---- embedding gather pipeline (fills x half of the staging) ----
        nc.gpsimd.load_library(mlp)
        gat = ctx.enter_context(tc.tile_pool(name="gather", bufs=2))
        trp = ctx.enter_context(tc.tile_pool(name="trp", bufs=2, space="PSUM"))
        xsg = ctx.enter_context(tc.tile_pool(name="xstage", bufs=3))
        sub = chunk // 128  # PE transposes per gather chunk
        for k in range(n_chunks):
            xr = gat.tile([P, sub, EP], f32, tag="xraw")
            if "gather" in skip:
                nc.vector.memset(xr[:], 0.0)
            else:
                nc.gpsimd.dma_gather(
                    xr[:],
                    emb_pad.ap(),
                    idx_sb[:, (chunk // 16) * k : (chunk // 16) * (k + 1)],
                    chunk,
                    chunk,
                    EP,
                )
            for c in range(sub):
                tp = trp.tile([EP, P], f32, tag="tp")
                nc.tensor.transpose(tp[:], xr[:, c, :], ident[:])
                xs = xsg.tile([EP, P], f32, tag="xs")
                nc.scalar.activation(xs[:], tp[:], AF.Copy)
                j0 = k * chunk + c * 128  # flat column position (no wrap)
                # crossbar DMA: partitions 0:64 -> 64:128 of the rhs staging
                nc.sync.dma_start(ring[64:128, j0 : j0 + 128], xs[:])

        # ---- the sequential scan ----
        psum = ctx.enter_context(tc.tile_pool(name="psum", bufs=2, space="PSUM"))
        sb = ctx.enter_context(tc.tile_pool(name="sb", bufs=2))
        for t in range(0 if "scan" in skip else t_steps):
            rb = t * bl
            rhs_t = ring[:, rb : rb + bl]
            ps_sig = psum.tile([H, 3 * bl], f32, tag="sig")
            ps_g = psum.tile([H, bl], f32, tag="g")
            ps_m = psum.tile([H, bl], f32, tag="m")
            # g first so tanh(g) can run on ACT ahead of the wide sigmoid
            nc.tensor.matmul(ps_g[:], lhs_sb[:, 128:192], rhs_t, start=True, stop=True)
            nc.tensor.matmul(ps_sig[:, 0:bl], lhs_sb[:, 0:64], rhs_t, start=True, stop=True)
            nc.tensor.matmul(ps_sig[:, bl : 2 * bl], lhs_sb[:, 64:128], rhs_t, start=True, stop=True)
            nc.tensor.matmul(ps_sig[:, 2 * bl : 3 * bl], lhs_sb[:, 192:256], rhs_t, start=True, stop=True)
            nc.tensor.matmul(ps_m[:], lhs_sb[:, 256:320], rhs_t, start=True, stop=True)

            gg = sb.tile([H, bl], f32, tag="gg")
            nc.scalar.activation(gg[:], ps_g[:], AF.Tanh)
            gsig = sb.tile([H, 3 * bl], f32, tag="gsig")
            # i,f sigmoid gates the chain; o's sigmoid runs off-chain (only
            # needed ~700ns later at the o*tanh(c) multiply)
            nc.scalar.activation(gsig[:, 0 : 2 * bl], ps_sig[:, 0 : 2 * bl], AF.Sigmoid)
            nc.scalar.activation(
                gsig[:, 2 * bl : 3 * bl], ps_sig[:, 2 * bl : 3 * bl], AF.Sigmoid
            )

            # (1-m)*h first: ready as soon as the mask matmul lands, fills
            # the DVE while ACT is still busy
            tmp4 = sb.tile([H, bl], f32, tag="tmp4")
            nc.vector.tensor_mul(tmp4[:], ps_m[:], ring[0:H, rb : rb + bl])

            tmp = sb.tile([H, bl], f32, tag="tmp")
            nc.vector.tensor_mul(tmp[:], gsig[:, 0:bl], gg[:])            # i * tanh(g)
            c_old = c_tiles[t % 2]
            c_new = c_tiles[(t + 1) % 2]
            tmp2 = sb.tile([H, bl], f32, tag="tmp2")
            nc.vector.tensor_mul(tmp2[:], gsig[:, bl : 2 * bl], c_old[:])  # f * c
            nc.vector.tensor_add(c_new[:], tmp[:], tmp2[:])

            th = sb.tile([H, bl], f32, tag="th")
            nc.scalar.activation(th[:], c_new[:], AF.Tanh)

            tmp3 = sb.tile([H, bl], f32, tag="tmp3")
            nc.vector.tensor_mul(tmp3[:], gsig[:, 2 * bl : 3 * bl], th[:])  # o * tanh(c)
            nb = (t + 1) * bl
            nc.vector.tensor_add(ring[0:H, nb : nb + bl], tmp3[:], tmp4[:])

        # ---- outputs: h2 history = staging rows 0:64, blocks 1..T ----
        n_out_dma = 8 if t_steps % 8 == 0 else 1
        span = t_steps * bl // n_out_dma
        for d in range(n_out_dma):
            nc.sync.dma_start(
                outT.ap()[:, d * span : (d + 1) * span],
                ring[0:H, bl + d * span : bl + (d + 1) * span],
            )
        nc.sync.dma_start(hTf.ap(), ring[0:H, t_steps * bl : (t_steps + 1) * bl])
        nc.sync.dma_start(cTf.ap(), c_tiles[t_steps % 2][:])

    nc.compile()
    return nc


def host_inputs(tokens, h0, c0, emb, W, U, b, t_steps=T, bl=BL, ncores=NCORES):
    """Build per-core input maps (pure layout prep: pad/concat/transpose/cast)."""
    f32 = np.float32
    emb_pad = np.zeros((V, EP), f32)
    emb_pad[:, :E] = np.asarray(emb, f32)
    emb_pad[:, E] = 1.0          # bias/ones column
    emb_pad[:, E + 1] = 0.0      # one-minus-mask column
    emb_pad[0, E + 1] = 1.0

    lhs = np.zeros((P, 5 * 64), f32)
    lhs[0:H, 0:G4] = np.asarray(U, f32)
    lhs[H : H + E, 0:G4] = np.asarray(W, f32)
    lhs[H + E, 0:G4] = np.asarray(b, f32)
    lhs[H + E + 1, 0:H] = -BIG            # i
    lhs[H + E + 1, H : 2 * H] = BIG       # f
    lhs[H + E + 1, 2 * H : 3 * H] = 0.0   # g
    lhs[H + E + 1, 3 * H : 4 * H] = -BIG  # o
    lhs[H + E + 1, 4 * H : 5 * H] = 1.0   # mask replicator column block

    in_maps = []
    for cid in range(ncores):
        tk = np.asarray(tokens[cid * bl : (cid + 1) * bl, :t_steps])
        flat = tk.T.reshape(-1)  # t-major: n = t*bl + b
        idxw = flat.reshape(-1, 16).T.astype(np.int16)   # [16, n/16], n = col*16+row
        idx_full = np.tile(idxw, (8, 1))                 # [128, n/16]
        in_maps.append(
            {
                "emb_pad": emb_pad,
                "idx_w": np.ascontiguousarray(idx_full),
                "lhs_all": lhs,
                "h0T": np.ascontiguousarray(np.asarray(h0, f32)[cid * bl : (cid + 1) * bl].T),
                "c0T": np.ascontiguousarray(np.asarray(c0, f32)[cid * bl : (cid + 1) * bl].T),
            }
        )
    return in_maps


def host_assemble(results, tokens, t_steps=T, bl=BL, ncores=NCORES):
    """Un-transpose shards, apply the masked-output carry fix, concat."""
    outs, hts, cts = [], [], []
    for cid in range(ncores):
        r = results[cid]
        hist = r["outT"].reshape(H, t_steps, bl).transpose(2, 1, 0)  # [b,t,h]
        tk = np.asarray(tokens[cid * bl : (cid + 1) * bl, :t_steps])
        m = tk != 0
        last = np.maximum.accumulate(
            np.where(m, np.arange(t_steps)[None, :], -1), axis=1
        )
        gathered = hist[np.arange(bl)[:, None], np.clip(last, 0, None)]
        out = np.where((last >= 0)[:, :, None], gathered, 0.0).astype(np.float32)
        outs.append(out)
        hts.append(r["hTf"].T.copy())
        cts.append(r["cTf"].T.copy())
    return (
        np.concatenate(outs, 0),
        np.concatenate(hts, 0),
        np.concatenate(cts, 0),
    )


def kernel(tokens, h0, c0, emb, W, U, b):
    from concourse.bass_utils import run_bass_kernel_spmd

    if "nc" not in _CACHE:
        _CACHE["nc"] = build_nc()
    nc = _CACHE["nc"]
    in_maps = host_inputs(tokens, h0, c0, emb, W, U, b)
    res = run_bass_kernel_spmd(nc, in_maps, list(range(NCORES)))
    return host_assemble(res.results, tokens)
